# revision 25
# baseline (speedup 1.0000x reference)
"""Trainium2 Bass kernel for 3rd-order HONU (nn_HONU_80865644249720).

out[b] = sum_{i<=j<=k} w_{ijk} * xb_i * xb_j * xb_k,  xb = [1, x] (65 feats)

Squaring-trick algorithm, batch-on-partitions accumulation layout:
  ps[p,b]   = (ES^T @ xbT)[p,b] = x_{j_p} + x_{k_p}     (TensorE, 9 tiles)
  ss[p,b]   = ps^2                                       (ACT Square / DVE mul)
  ztT[b,i]  = sum_p ss[p,b] (W[p,i]/2)  -  sum_m xbT[m,b]^2 G[m,i]
              (36+4 small matmuls, N=65 cols each: batch on PSUM partitions)
  out[b]    = sum_i ztT[b,i] * xb[b,i]                   (DVE tensor_tensor_reduce)

Sharding: 4-way batch (512 rows) x 2-way pairs (9 tiles) over 8 cores.
Each core computes a partial out over its pair half; host sums halves.
Inputs land in bf16 via 3 consolidated DMAs (2 HWDGE + 1 SWDGE);
the [128,4] result returns via one small HWDGE DMA.
"""

import os

import numpy as np

IN_FEATURES = 64
NF = IN_FEATURES + 1  # 65 features incl. bias
BATCH = 2048
N_CORES = 8
NBS = 4  # batch shards
NPS = 2  # pair shards
BC = BATCH // NBS  # 512 batch rows per core
NBLK = BC // 128  # 4 batch blocks of 128
NPAIR = NF * (NF + 1) // 2  # 2145
PH = 9  # pair tiles per half
PCOLS = PH * 128  # 1152 pair columns per core
PPAD = 2 * PCOLS  # 2304
DVE_TILES = (1, 3, 5)  # tiles squared on VectorE; rest on ScalarE

_CACHE = {}

# Stashed BassKernelResults from the most recent run (for test.py timing).
LAST_RESULTS = None


def _pair_maps():
    """j_p, k_p for the j-major triangular pair ordering."""
    jp = np.concatenate([np.full(NF - j, j, np.int64) for j in range(NF)])
    kp = np.concatenate([np.arange(j, NF, dtype=np.int64) for j in range(NF)])
    return jp, kp


def _build_bass():
    import concourse.bacc as bacc
    import concourse.mybir as mybir
    import concourse.tile as tile

    f32 = mybir.dt.float32
    bf16 = mybir.dt.bfloat16
    Square = mybir.ActivationFunctionType.Square
    add = mybir.AluOpType.add

    nc = bacc.Bacc(
        target_bir_lowering=False,
        debug=False,
        enable_asserts=False,
        num_devices=N_CORES,
    )

    # a1 = xbt[65,512] ++ es tiles 0..3 [65,512]
    a1_d = nc.dram_tensor("a1", [NF, 1024], bf16, kind="ExternalInput").ap()
    # a2 = es tiles 4..8 [65,640] ++ (-G) [65,65]
    a2_d = nc.dram_tensor("a2", [NF, 705], bf16, kind="ExternalInput").ap()
    # b = wh tiles [128, 9*65] ++ xbB blocks [128, 4*65]
    b_d = nc.dram_tensor("b", [128, 845], bf16, kind="ExternalInput").ap()
    out_d = nc.dram_tensor("out", [128, NBLK], f32, kind="ExternalOutput").ap()

    from contextlib import ExitStack

    with tile.TileContext(nc) as tc, ExitStack() as ctx:
        consts = ctx.enter_context(tc.tile_pool(name="consts", bufs=1))
        ss_pool = ctx.enter_context(tc.tile_pool(name="ss", bufs=PH))
        psc_pool = ctx.enter_context(tc.tile_pool(name="psc", bufs=2))
        ps_pool = ctx.enter_context(tc.tile_pool(name="ps", bufs=6, space="PSUM"))
        zt_pool = ctx.enter_context(tc.tile_pool(name="zt", bufs=1, space="PSUM"))

        ta1 = consts.tile([NF, 1024], bf16, tag="ta1")
        ta2 = consts.tile([NF, 705], bf16, tag="ta2")
        tb = consts.tile([128, 845], bf16, tag="tb")
        sq = consts.tile([NF, BC], bf16, tag="sq")
        outsb = consts.tile([128, NBLK], f32, tag="outsb")

        nc.sync.dma_start(ta1[:], a1_d)
        nc.sync.dma_start(ta2[:], a2_d)
        nc.gpsimd.dma_start(tb[:], b_d)

        xbt = ta1[:, 0:BC]  # [65, 512] bf16

        # sq = xbt^2 feeds the G-correction matmuls (bf16 SBUF 2x path).
        nc.vector.tensor_mul(sq[:], xbt, xbt)

        # 4 per-block accumulators, 260B each (packed into one PSUM bank).
        # Two accumulator tiles, 2 blocks each (PSUM allocation is
        # bank-granular; pairing blocks keeps it to 2 banks).
        zt01 = zt_pool.tile([128, 2 * NF], f32, tag="zt01")
        zt23 = zt_pool.tile([128, 2 * NF], f32, tag="zt23")
        ztq = [zt01[:, 0:NF], zt01[:, NF : 2 * NF],
               zt23[:, 0:NF], zt23[:, NF : 2 * NF]]

        # Sels first (7 ps bufs) so PE never queues behind a stalled acc.
        ps_t = []
        ss_t = []

        def emit_sel(t):
            ps = ps_pool.tile([128, BC], f32, tag="ps")
            if t < 4:
                es_v = ta1[:, BC + t * 128 : BC + (t + 1) * 128]
            else:
                es_v = ta2[:, (t - 4) * 128 : (t - 3) * 128]
            nc.tensor.matmul(ps[:], es_v, xbt, start=True, stop=True)
            ps_t.append(ps)

        def emit_square(t):
            ss = ss_pool.tile([128, BC], bf16, tag="ss")
            if t in DVE_TILES:
                # DVE cannot dual-read PSUM: copy to SBUF bf16, then 2x mul.
                psc = psc_pool.tile([128, BC], bf16, tag="psc")
                nc.vector.tensor_copy(psc[:], ps_t[t][:])
                nc.vector.tensor_mul(ss[:], psc[:], psc[:])
            else:
                nc.scalar.activation(ss[:], ps_t[t][:], Square)
            ss_t.append(ss)

        def emit_accs(t):
            for q in range(NBLK):
                nc.tensor.matmul(
                    ztq[q],
                    ss_t[t][:, q * 128 : (q + 1) * 128],
                    tb[:, t * NF : (t + 1) * NF],
                    start=(t == 0 and q % 2 == 0),
                    stop=False,
                    skip_group_check=True,
                )

        gneg = ta2[:, 640:705]

        for t in range(7):
            emit_sel(t)
        for t in range(7):
            emit_square(t)
            if t + 7 < PH:
                emit_sel(t + 7)
            emit_accs(t)
            if t == 0:
                # G matmuls need only sq: emit them right after the first
                # accs so no PE work besides the t7/t8 closers remains at
                # the end of the square pipeline.
                for q in range(NBLK):
                    nc.tensor.matmul(
                        ztq[q],
                        sq[:, q * 128 : (q + 1) * 128],
                        gneg,
                        start=False,
                        stop=False,
                        skip_group_check=True,
                    )
        for t in range(7, PH - 1):
            emit_square(t)
            emit_accs(t)
        # Last tile's accs close each block's accumulation group.
        emit_square(PH - 1)
        for q in range(NBLK):
            nc.tensor.matmul(
                ztq[q],
                ss_t[PH - 1][:, q * 128 : (q + 1) * 128],
                tb[:, (PH - 1) * NF : PH * NF],
                start=False,
                stop=True,
                skip_group_check=True,
            )

        # out[b] = sum_i ztT[b,i] * xb[b,i]: per zt tile, one elementwise
        # mul then one grouped (3D-AP) free-axis reduce into 2 output cols.
        for half, zt_tile in ((0, zt01), (1, zt23)):
            fm = consts.tile([128, 2 * NF], bf16, tag=f"fm{half}")
            nc.vector.tensor_mul(
                fm[:],
                zt_tile[:],
                tb[:, (PH + 2 * half) * NF : (PH + 2 * half + 2) * NF],
            )
            nc.vector.tensor_reduce(
                outsb[:, 2 * half : 2 * half + 2],
                fm[:].rearrange("p (g x) -> p g x", g=2),
                mybir.AxisListType.X,
                add,
            )

        nc.sync.dma_start(out_d, outsb[:])

    nc.compile()
    return nc


def _host_prep(x, weights, comb_idx):
    """Build per-core input maps (numpy only, O(NPAIR) work)."""
    import ml_dtypes

    bf16 = ml_dtypes.bfloat16
    jp, kp = _pair_maps()

    # Fused selection matrix ES (diag pairs get 2.0).
    cols = np.arange(NPAIR)
    es = np.zeros((NF, PPAD), np.float32)
    np.add.at(es, (jp, cols), 1.0)
    np.add.at(es, (kp, cols), 1.0)

    # W2[i, pidx(j,k)] = w_{ijk}; comb rows are sorted triples (i<=j<=k).
    ci = np.asarray(comb_idx, np.int64)
    c0, c1, c2 = ci[:, 0], ci[:, 1], ci[:, 2]
    pcol = c1 * NF - (c1 * (c1 - 1)) // 2 + (c2 - c1)
    w2 = np.zeros((NF, PPAD), np.float32)
    w2[c0, pcol] = np.asarray(weights, np.float32)

    # Per-half x^2 correction G_h[m, i] (negated: ztT += sqT @ (-G)).
    wp = w2.T[:NPAIR]  # [p, i]
    od = jp != kp
    coefj = np.where(od, 0.5, 1.0)
    half_id = (np.arange(NPAIR) >= PCOLS).astype(np.int64)

    xb = np.concatenate(
        [np.ones((BATCH, 1), np.float32), np.asarray(x, np.float32)], axis=1
    )

    a1_h, a2_h, b_h = [], [], []
    for h in range(NPS):
        sl = slice(h * PCOLS, (h + 1) * PCOLS)
        es_h = es[:, sl]
        gm = np.zeros((NF, NF), np.float32)  # [m, i]
        mask = half_id == h
        np.add.at(gm, jp[mask], coefj[mask, None] * wp[mask])
        np.add.at(gm, kp[mask & od], 0.5 * wp[mask & od])
        # wh tiles [128, 65] per pair tile, 0.5*W
        wh = (0.5 * w2[:, sl].T).reshape(PH, 128, NF)
        wh_flat = wh.transpose(1, 0, 2).reshape(128, PH * NF)
        a2_h.append(
            np.ascontiguousarray(
                np.concatenate([es_h[:, 512:], -gm], axis=1)
            ).astype(bf16)
        )
        a1_es = es_h[:, :512]
        a1_h.append(a1_es)  # xbt prepended per batch shard below
        b_h.append(wh_flat)

    a1_maps, b_maps = {}, {}
    for q in range(NBS):
        rows = xb[q * BC : (q + 1) * BC]  # [512, 65]
        xbt = rows.T  # [65, 512]
        xbB = rows.reshape(NBLK, 128, NF).transpose(1, 0, 2).reshape(128, NBLK * NF)
        for h in range(NPS):
            a1_maps[(q, h)] = np.ascontiguousarray(
                np.concatenate([xbt, a1_h[h]], axis=1)
            ).astype(bf16)
            b_maps[(q, h)] = np.ascontiguousarray(
                np.concatenate([b_h[h], xbB], axis=1).astype(bf16)
            )

    in_maps = []
    for c in range(N_CORES):
        q, h = c % NBS, c // NBS
        in_maps.append(
            {"a1": a1_maps[(q, h)], "a2": a2_h[h], "b": b_maps[(q, h)]}
        )
    return in_maps


def kernel(x, weights, comb_idx):
    global LAST_RESULTS
    from concourse import bass_utils

    if "nc" not in _CACHE:
        _CACHE["nc"] = _build_bass()
    nc = _CACHE["nc"]

    in_maps = _host_prep(x, weights, comb_idx)
    res = bass_utils.run_bass_kernel_spmd(
        nc,
        in_maps,
        core_ids=list(range(N_CORES)),
        trace=bool(int(os.environ.get("HONU_TRACE", "0"))),
    )
    LAST_RESULTS = res
    # out[p, q] = batch-local row q*128+p of this core's 512-row chunk
    parts = [
        np.asarray(r["out"], np.float32).T.reshape(BC)
        for r in res.results
    ]
    out = np.concatenate([parts[q] + parts[q + NBS] for q in range(NBS)])
    return out.reshape(BATCH, 1).astype(np.float32)


# revision 29
# speedup vs baseline: 1.0036x; 1.0036x over previous
"""Trainium2 Bass kernel for 3rd-order HONU (nn_HONU_80865644249720).

out[b] = sum_{i<=j<=k} w_{ijk} * xb_i * xb_j * xb_k,  xb = [1, x] (65 feats)

Squaring-trick algorithm, batch-on-partitions accumulation layout:
  ps[p,b]   = (ES^T @ xbT)[p,b] = x_{j_p} + x_{k_p}     (TensorE, 9 tiles)
  ss[p,b]   = ps^2                                       (ACT Square / DVE mul)
  ztT[b,i]  = sum_p ss[p,b] (W[p,i]/2)  -  sum_m xbT[m,b]^2 G[m,i]
              (36+4 small matmuls, N=65 cols each: batch on PSUM partitions)
  out[b]    = sum_i ztT[b,i] * xb[b,i]                   (DVE tensor_tensor_reduce)

Sharding: 4-way batch (512 rows) x 2-way pairs (9 tiles) over 8 cores.
Each core computes a partial out over its pair half; host sums halves.
Inputs land in bf16 via 3 consolidated DMAs (2 HWDGE + 1 SWDGE);
the [128,4] result returns via one small HWDGE DMA.
"""

import os

import numpy as np

IN_FEATURES = 64
NF = IN_FEATURES + 1  # 65 features incl. bias
BATCH = 2048
N_CORES = 8
NBS = 4  # batch shards
NPS = 2  # pair shards
BC = BATCH // NBS  # 512 batch rows per core
NBLK = BC // 128  # 4 batch blocks of 128
NPAIR = NF * (NF + 1) // 2  # 2145
PH = 9  # pair tiles per half
PCOLS = PH * 128  # 1152 pair columns per core
PPAD = 2 * PCOLS  # 2304
DVE_TILES = (1, 4, 6)  # tiles squared on VectorE; rest on ScalarE

_CACHE = {}

# Stashed BassKernelResults from the most recent run (for test.py timing).
LAST_RESULTS = None


def _pair_maps():
    """j_p, k_p for the j-major triangular pair ordering."""
    jp = np.concatenate([np.full(NF - j, j, np.int64) for j in range(NF)])
    kp = np.concatenate([np.arange(j, NF, dtype=np.int64) for j in range(NF)])
    return jp, kp


def _build_bass():
    import concourse.bacc as bacc
    import concourse.mybir as mybir
    import concourse.tile as tile

    f32 = mybir.dt.float32
    bf16 = mybir.dt.bfloat16
    Square = mybir.ActivationFunctionType.Square
    add = mybir.AluOpType.add

    nc = bacc.Bacc(
        target_bir_lowering=False,
        debug=False,
        enable_asserts=False,
        num_devices=N_CORES,
    )

    # a1 = xbt[65,512] ++ es tiles 0..3 [65,512]
    a1_d = nc.dram_tensor("a1", [NF, 1024], bf16, kind="ExternalInput").ap()
    # a2 = es tiles 4..8 [65,640] ++ (-G) [65,65]
    a2_d = nc.dram_tensor("a2", [NF, 705], bf16, kind="ExternalInput").ap()
    # b = wh tiles [128, 9*65] ++ xbB blocks [128, 4*65]
    b_d = nc.dram_tensor("b", [128, 845], bf16, kind="ExternalInput").ap()
    out_d = nc.dram_tensor("out", [128, NBLK], f32, kind="ExternalOutput").ap()

    from contextlib import ExitStack

    with tile.TileContext(nc) as tc, ExitStack() as ctx:
        consts = ctx.enter_context(tc.tile_pool(name="consts", bufs=1))
        ss_pool = ctx.enter_context(tc.tile_pool(name="ss", bufs=PH))
        psc_pool = ctx.enter_context(tc.tile_pool(name="psc", bufs=2))
        ps_pool = ctx.enter_context(tc.tile_pool(name="ps", bufs=6, space="PSUM"))
        zt_pool = ctx.enter_context(tc.tile_pool(name="zt", bufs=1, space="PSUM"))

        ta1 = consts.tile([NF, 1024], bf16, tag="ta1")
        ta2 = consts.tile([NF, 705], bf16, tag="ta2")
        tb = consts.tile([128, 845], bf16, tag="tb")
        sq = consts.tile([NF, BC], bf16, tag="sq")
        outsb = consts.tile([128, NBLK], f32, tag="outsb")

        nc.sync.dma_start(ta1[:], a1_d)
        nc.sync.dma_start(ta2[:], a2_d)
        nc.gpsimd.dma_start(tb[:], b_d)

        xbt = ta1[:, 0:BC]  # [65, 512] bf16

        # sq = xbt^2 feeds the G-correction matmuls (bf16 SBUF 2x path).
        nc.vector.tensor_mul(sq[:], xbt, xbt)

        # 4 per-block accumulators, 260B each (packed into one PSUM bank).
        # Two accumulator tiles, 2 blocks each (PSUM allocation is
        # bank-granular; pairing blocks keeps it to 2 banks).
        zt01 = zt_pool.tile([128, 2 * NF], f32, tag="zt01")
        zt23 = zt_pool.tile([128, 2 * NF], f32, tag="zt23")
        ztq = [zt01[:, 0:NF], zt01[:, NF : 2 * NF],
               zt23[:, 0:NF], zt23[:, NF : 2 * NF]]

        # Sels first (7 ps bufs) so PE never queues behind a stalled acc.
        ps_t = []
        ss_t = []

        def emit_sel(t):
            ps = ps_pool.tile([128, BC], f32, tag="ps")
            if t < 4:
                es_v = ta1[:, BC + t * 128 : BC + (t + 1) * 128]
            else:
                es_v = ta2[:, (t - 4) * 128 : (t - 3) * 128]
            nc.tensor.matmul(ps[:], es_v, xbt, start=True, stop=True)
            ps_t.append(ps)

        def emit_square(t):
            ss = ss_pool.tile([128, BC], bf16, tag="ss")
            if t in DVE_TILES:
                # DVE cannot dual-read PSUM: copy to SBUF bf16, then 2x mul.
                psc = psc_pool.tile([128, BC], bf16, tag="psc")
                nc.vector.tensor_copy(psc[:], ps_t[t][:])
                nc.vector.tensor_mul(ss[:], psc[:], psc[:])
            else:
                nc.scalar.activation(ss[:], ps_t[t][:], Square)
            ss_t.append(ss)

        def emit_accs(t):
            for q in range(NBLK):
                nc.tensor.matmul(
                    ztq[q],
                    ss_t[t][:, q * 128 : (q + 1) * 128],
                    tb[:, t * NF : (t + 1) * NF],
                    start=(t == 0 and q % 2 == 0),
                    stop=False,
                    skip_group_check=True,
                )

        gneg = ta2[:, 640:705]

        for t in range(7):
            emit_sel(t)
        for t in range(7):
            emit_square(t)
            if t + 7 < PH:
                emit_sel(t + 7)
            emit_accs(t)
            if t == 0:
                # G matmuls need only sq: emit them right after the first
                # accs so no PE work besides the t7/t8 closers remains at
                # the end of the square pipeline.
                for q in range(NBLK):
                    nc.tensor.matmul(
                        ztq[q],
                        sq[:, q * 128 : (q + 1) * 128],
                        gneg,
                        start=False,
                        stop=False,
                        skip_group_check=True,
                    )
        for t in range(7, PH - 1):
            emit_square(t)
            emit_accs(t)
        # Last tile's accs close each block's accumulation group.
        emit_square(PH - 1)
        for q in range(NBLK):
            nc.tensor.matmul(
                ztq[q],
                ss_t[PH - 1][:, q * 128 : (q + 1) * 128],
                tb[:, (PH - 1) * NF : PH * NF],
                start=False,
                stop=True,
                skip_group_check=True,
            )

        # out[b] = sum_i ztT[b,i] * xb[b,i]: per zt tile, one elementwise
        # mul then one grouped (3D-AP) free-axis reduce into 2 output cols.
        for half, zt_tile in ((0, zt01), (1, zt23)):
            fm = consts.tile([128, 2 * NF], bf16, tag=f"fm{half}")
            nc.vector.tensor_mul(
                fm[:],
                zt_tile[:],
                tb[:, (PH + 2 * half) * NF : (PH + 2 * half + 2) * NF],
            )
            nc.vector.tensor_reduce(
                outsb[:, 2 * half : 2 * half + 2],
                fm[:].rearrange("p (g x) -> p g x", g=2),
                mybir.AxisListType.X,
                add,
            )

        nc.sync.dma_start(out_d, outsb[:])

    nc.compile()
    return nc


def _host_prep(x, weights, comb_idx):
    """Build per-core input maps (numpy only, O(NPAIR) work)."""
    import ml_dtypes

    bf16 = ml_dtypes.bfloat16
    jp, kp = _pair_maps()

    # Fused selection matrix ES (diag pairs get 2.0).
    cols = np.arange(NPAIR)
    es = np.zeros((NF, PPAD), np.float32)
    np.add.at(es, (jp, cols), 1.0)
    np.add.at(es, (kp, cols), 1.0)

    # W2[i, pidx(j,k)] = w_{ijk}; comb rows are sorted triples (i<=j<=k).
    ci = np.asarray(comb_idx, np.int64)
    c0, c1, c2 = ci[:, 0], ci[:, 1], ci[:, 2]
    pcol = c1 * NF - (c1 * (c1 - 1)) // 2 + (c2 - c1)
    w2 = np.zeros((NF, PPAD), np.float32)
    w2[c0, pcol] = np.asarray(weights, np.float32)

    # Per-half x^2 correction G_h[m, i] (negated: ztT += sqT @ (-G)).
    wp = w2.T[:NPAIR]  # [p, i]
    od = jp != kp
    coefj = np.where(od, 0.5, 1.0)
    half_id = (np.arange(NPAIR) >= PCOLS).astype(np.int64)

    xb = np.concatenate(
        [np.ones((BATCH, 1), np.float32), np.asarray(x, np.float32)], axis=1
    )

    a1_h, a2_h, b_h = [], [], []
    for h in range(NPS):
        sl = slice(h * PCOLS, (h + 1) * PCOLS)
        es_h = es[:, sl]
        gm = np.zeros((NF, NF), np.float32)  # [m, i]
        mask = half_id == h
        np.add.at(gm, jp[mask], coefj[mask, None] * wp[mask])
        np.add.at(gm, kp[mask & od], 0.5 * wp[mask & od])
        # wh tiles [128, 65] per pair tile, 0.5*W
        wh = (0.5 * w2[:, sl].T).reshape(PH, 128, NF)
        wh_flat = wh.transpose(1, 0, 2).reshape(128, PH * NF)
        a2_h.append(
            np.ascontiguousarray(
                np.concatenate([es_h[:, 512:], -gm], axis=1)
            ).astype(bf16)
        )
        a1_es = es_h[:, :512]
        a1_h.append(a1_es)  # xbt prepended per batch shard below
        b_h.append(wh_flat)

    a1_maps, b_maps = {}, {}
    for q in range(NBS):
        rows = xb[q * BC : (q + 1) * BC]  # [512, 65]
        xbt = rows.T  # [65, 512]
        xbB = rows.reshape(NBLK, 128, NF).transpose(1, 0, 2).reshape(128, NBLK * NF)
        for h in range(NPS):
            a1_maps[(q, h)] = np.ascontiguousarray(
                np.concatenate([xbt, a1_h[h]], axis=1)
            ).astype(bf16)
            b_maps[(q, h)] = np.ascontiguousarray(
                np.concatenate([b_h[h], xbB], axis=1).astype(bf16)
            )

    in_maps = []
    for c in range(N_CORES):
        q, h = c % NBS, c // NBS
        in_maps.append(
            {"a1": a1_maps[(q, h)], "a2": a2_h[h], "b": b_maps[(q, h)]}
        )
    return in_maps


def kernel(x, weights, comb_idx):
    global LAST_RESULTS
    from concourse import bass_utils

    if "nc" not in _CACHE:
        _CACHE["nc"] = _build_bass()
    nc = _CACHE["nc"]

    in_maps = _host_prep(x, weights, comb_idx)
    res = bass_utils.run_bass_kernel_spmd(
        nc,
        in_maps,
        core_ids=list(range(N_CORES)),
        trace=bool(int(os.environ.get("HONU_TRACE", "0"))),
    )
    LAST_RESULTS = res
    # out[p, q] = batch-local row q*128+p of this core's 512-row chunk
    parts = [
        np.asarray(r["out"], np.float32).T.reshape(BC)
        for r in res.results
    ]
    out = np.concatenate([parts[q] + parts[q + NBS] for q in range(NBS)])
    return out.reshape(BATCH, 1).astype(np.float32)


# revision 32
# speedup vs baseline: 1.0076x; 1.0040x over previous
"""Trainium2 Bass kernel for 3rd-order HONU (nn_HONU_80865644249720).

out[b] = sum_{i<=j<=k} w_{ijk} * xb_i * xb_j * xb_k,  xb = [1, x] (65 feats)

Squaring-trick algorithm, batch-on-partitions accumulation layout:
  ps[p,b]   = (ES^T @ xbT)[p,b] = x_{j_p} + x_{k_p}     (TensorE, 9 tiles)
  ss[p,b]   = ps^2                                       (ACT Square / DVE mul)
  ztT[b,i]  = sum_p ss[p,b] (W[p,i]/2)  -  sum_m xbT[m,b]^2 G[m,i]
              (36+4 small matmuls, N=65 cols each: batch on PSUM partitions)
  out[b]    = sum_i ztT[b,i] * xb[b,i]                   (DVE tensor_tensor_reduce)

Sharding: 4-way batch (512 rows) x 2-way pairs (9 tiles) over 8 cores.
Each core computes a partial out over its pair half; host sums halves.
Inputs land in bf16 via 3 consolidated DMAs (2 HWDGE + 1 SWDGE);
the [128,4] result returns via one small HWDGE DMA.
"""

import os

import numpy as np

IN_FEATURES = 64
NF = IN_FEATURES + 1  # 65 features incl. bias
BATCH = 2048
N_CORES = 8
NBS = 4  # batch shards
NPS = 2  # pair shards
BC = BATCH // NBS  # 512 batch rows per core
NBLK = BC // 128  # 4 batch blocks of 128
NPAIR = NF * (NF + 1) // 2  # 2145
PH = 9  # pair tiles per half
PCOLS = PH * 128  # 1152 pair columns per core
PPAD = 2 * PCOLS  # 2304
DVE_TILES = (1, 4, 6)  # tiles squared on VectorE; rest on ScalarE

_CACHE = {}

# Stashed BassKernelResults from the most recent run (for test.py timing).
LAST_RESULTS = None


def _pair_maps():
    """j_p, k_p for the j-major triangular pair ordering."""
    jp = np.concatenate([np.full(NF - j, j, np.int64) for j in range(NF)])
    kp = np.concatenate([np.arange(j, NF, dtype=np.int64) for j in range(NF)])
    return jp, kp


def _build_bass():
    import concourse.bacc as bacc
    import concourse.mybir as mybir
    import concourse.tile as tile

    f32 = mybir.dt.float32
    bf16 = mybir.dt.bfloat16
    Square = mybir.ActivationFunctionType.Square
    add = mybir.AluOpType.add

    nc = bacc.Bacc(
        target_bir_lowering=False,
        debug=False,
        enable_asserts=False,
        num_devices=N_CORES,
    )

    # a1 = xbt[65,512] ++ es tiles 0..2 [65,384] (shorter first transfer)
    a1_d = nc.dram_tensor("a1", [NF, 896], bf16, kind="ExternalInput").ap()
    # a2 = es tiles 3..8 [65,768] ++ (-G) [65,65]
    a2_d = nc.dram_tensor("a2", [NF, 833], bf16, kind="ExternalInput").ap()
    # b = wh tiles [128, 9*65] ++ xbB blocks [128, 4*65]
    b_d = nc.dram_tensor("b", [128, 845], bf16, kind="ExternalInput").ap()
    out_d = nc.dram_tensor("out", [128, NBLK], f32, kind="ExternalOutput").ap()

    from contextlib import ExitStack

    with tile.TileContext(nc) as tc, ExitStack() as ctx:
        consts = ctx.enter_context(tc.tile_pool(name="consts", bufs=1))
        ss_pool = ctx.enter_context(tc.tile_pool(name="ss", bufs=PH))
        psc_pool = ctx.enter_context(tc.tile_pool(name="psc", bufs=2))
        ps_pool = ctx.enter_context(tc.tile_pool(name="ps", bufs=6, space="PSUM"))
        zt_pool = ctx.enter_context(tc.tile_pool(name="zt", bufs=1, space="PSUM"))

        ta1 = consts.tile([NF, 896], bf16, tag="ta1")
        ta2 = consts.tile([NF, 833], bf16, tag="ta2")
        tb = consts.tile([128, 845], bf16, tag="tb")
        sq = consts.tile([NF, BC], bf16, tag="sq")
        outsb = consts.tile([128, NBLK], f32, tag="outsb")

        nc.sync.dma_start(ta1[:], a1_d)
        nc.sync.dma_start(ta2[:], a2_d)
        nc.gpsimd.dma_start(tb[:], b_d)

        xbt = ta1[:, 0:BC]  # [65, 512] bf16

        # sq = xbt^2 feeds the G-correction matmuls (bf16 SBUF 2x path).
        nc.vector.tensor_mul(sq[:], xbt, xbt)

        # 4 per-block accumulators, 260B each (packed into one PSUM bank).
        # Two accumulator tiles, 2 blocks each (PSUM allocation is
        # bank-granular; pairing blocks keeps it to 2 banks).
        zt01 = zt_pool.tile([128, 2 * NF], f32, tag="zt01")
        zt23 = zt_pool.tile([128, 2 * NF], f32, tag="zt23")
        ztq = [zt01[:, 0:NF], zt01[:, NF : 2 * NF],
               zt23[:, 0:NF], zt23[:, NF : 2 * NF]]

        # Sels first (7 ps bufs) so PE never queues behind a stalled acc.
        ps_t = []
        ss_t = []

        def emit_sel(t):
            ps = ps_pool.tile([128, BC], f32, tag="ps")
            if t < 3:
                es_v = ta1[:, BC + t * 128 : BC + (t + 1) * 128]
            else:
                es_v = ta2[:, (t - 3) * 128 : (t - 2) * 128]
            nc.tensor.matmul(ps[:], es_v, xbt, start=True, stop=True)
            ps_t.append(ps)

        def emit_square(t):
            ss = ss_pool.tile([128, BC], bf16, tag="ss")
            if t in DVE_TILES:
                # DVE cannot dual-read PSUM: copy to SBUF bf16, then 2x mul.
                psc = psc_pool.tile([128, BC], bf16, tag="psc")
                nc.vector.tensor_copy(psc[:], ps_t[t][:])
                nc.vector.tensor_mul(ss[:], psc[:], psc[:])
            else:
                nc.scalar.activation(ss[:], ps_t[t][:], Square)
            ss_t.append(ss)

        def emit_accs(t):
            for q in range(NBLK):
                nc.tensor.matmul(
                    ztq[q],
                    ss_t[t][:, q * 128 : (q + 1) * 128],
                    tb[:, t * NF : (t + 1) * NF],
                    start=(t == 0 and q % 2 == 0),
                    stop=False,
                    skip_group_check=True,
                )

        gneg = ta2[:, 768:833]

        for t in range(7):
            emit_sel(t)
        for t in range(7):
            emit_square(t)
            if t + 7 < PH:
                emit_sel(t + 7)
            emit_accs(t)
            if t == 0:
                # G matmuls need only sq: emit them right after the first
                # accs so no PE work besides the t7/t8 closers remains at
                # the end of the square pipeline.
                for q in range(NBLK):
                    nc.tensor.matmul(
                        ztq[q],
                        sq[:, q * 128 : (q + 1) * 128],
                        gneg,
                        start=False,
                        stop=False,
                        skip_group_check=True,
                    )
        for t in range(7, PH - 1):
            emit_square(t)
            emit_accs(t)
        # Last tile's accs close each block's accumulation group.
        emit_square(PH - 1)
        for q in range(NBLK):
            nc.tensor.matmul(
                ztq[q],
                ss_t[PH - 1][:, q * 128 : (q + 1) * 128],
                tb[:, (PH - 1) * NF : PH * NF],
                start=False,
                stop=True,
                skip_group_check=True,
            )

        # out[b] = sum_i ztT[b,i] * xb[b,i]: per zt tile, one elementwise
        # mul then one grouped (3D-AP) free-axis reduce into 2 output cols.
        for half, zt_tile in ((0, zt01), (1, zt23)):
            fm = consts.tile([128, 2 * NF], bf16, tag=f"fm{half}")
            nc.vector.tensor_mul(
                fm[:],
                zt_tile[:],
                tb[:, (PH + 2 * half) * NF : (PH + 2 * half + 2) * NF],
            )
            nc.vector.tensor_reduce(
                outsb[:, 2 * half : 2 * half + 2],
                fm[:].rearrange("p (g x) -> p g x", g=2),
                mybir.AxisListType.X,
                add,
            )

        nc.sync.dma_start(out_d, outsb[:])

    nc.compile()
    return nc


def _host_prep(x, weights, comb_idx):
    """Build per-core input maps (numpy only, O(NPAIR) work)."""
    import ml_dtypes

    bf16 = ml_dtypes.bfloat16
    jp, kp = _pair_maps()

    # Fused selection matrix ES (diag pairs get 2.0).
    cols = np.arange(NPAIR)
    es = np.zeros((NF, PPAD), np.float32)
    np.add.at(es, (jp, cols), 1.0)
    np.add.at(es, (kp, cols), 1.0)

    # W2[i, pidx(j,k)] = w_{ijk}; comb rows are sorted triples (i<=j<=k).
    ci = np.asarray(comb_idx, np.int64)
    c0, c1, c2 = ci[:, 0], ci[:, 1], ci[:, 2]
    pcol = c1 * NF - (c1 * (c1 - 1)) // 2 + (c2 - c1)
    w2 = np.zeros((NF, PPAD), np.float32)
    w2[c0, pcol] = np.asarray(weights, np.float32)

    # Per-half x^2 correction G_h[m, i] (negated: ztT += sqT @ (-G)).
    wp = w2.T[:NPAIR]  # [p, i]
    od = jp != kp
    coefj = np.where(od, 0.5, 1.0)
    half_id = (np.arange(NPAIR) >= PCOLS).astype(np.int64)

    xb = np.concatenate(
        [np.ones((BATCH, 1), np.float32), np.asarray(x, np.float32)], axis=1
    )

    a1_h, a2_h, b_h = [], [], []
    for h in range(NPS):
        sl = slice(h * PCOLS, (h + 1) * PCOLS)
        es_h = es[:, sl]
        gm = np.zeros((NF, NF), np.float32)  # [m, i]
        mask = half_id == h
        np.add.at(gm, jp[mask], coefj[mask, None] * wp[mask])
        np.add.at(gm, kp[mask & od], 0.5 * wp[mask & od])
        # wh tiles [128, 65] per pair tile, 0.5*W
        wh = (0.5 * w2[:, sl].T).reshape(PH, 128, NF)
        wh_flat = wh.transpose(1, 0, 2).reshape(128, PH * NF)
        a2_h.append(
            np.ascontiguousarray(
                np.concatenate([es_h[:, 384:], -gm], axis=1)
            ).astype(bf16)
        )
        a1_es = es_h[:, :384]
        a1_h.append(a1_es)  # xbt prepended per batch shard below
        b_h.append(wh_flat)

    a1_maps, b_maps = {}, {}
    for q in range(NBS):
        rows = xb[q * BC : (q + 1) * BC]  # [512, 65]
        xbt = rows.T  # [65, 512]
        xbB = rows.reshape(NBLK, 128, NF).transpose(1, 0, 2).reshape(128, NBLK * NF)
        for h in range(NPS):
            a1_maps[(q, h)] = np.ascontiguousarray(
                np.concatenate([xbt, a1_h[h]], axis=1)
            ).astype(bf16)
            b_maps[(q, h)] = np.ascontiguousarray(
                np.concatenate([b_h[h], xbB], axis=1).astype(bf16)
            )

    in_maps = []
    for c in range(N_CORES):
        q, h = c % NBS, c // NBS
        in_maps.append(
            {"a1": a1_maps[(q, h)], "a2": a2_h[h], "b": b_maps[(q, h)]}
        )
    return in_maps


def kernel(x, weights, comb_idx):
    global LAST_RESULTS
    from concourse import bass_utils

    if "nc" not in _CACHE:
        _CACHE["nc"] = _build_bass()
    nc = _CACHE["nc"]

    in_maps = _host_prep(x, weights, comb_idx)
    res = bass_utils.run_bass_kernel_spmd(
        nc,
        in_maps,
        core_ids=list(range(N_CORES)),
        trace=bool(int(os.environ.get("HONU_TRACE", "0"))),
    )
    LAST_RESULTS = res
    # out[p, q] = batch-local row q*128+p of this core's 512-row chunk
    parts = [
        np.asarray(r["out"], np.float32).T.reshape(BC)
        for r in res.results
    ]
    out = np.concatenate([parts[q] + parts[q + NBS] for q in range(NBS)])
    return out.reshape(BATCH, 1).astype(np.float32)
